# revision 18
# baseline (speedup 1.0000x reference)
"""Trainium2 Bass kernel for nn_CATLayer (moe_routing).

Reference computation:
  - per-expert FFN over all E=8 experts:  a_e = relu(x @ W1_e + b1_e) @ W2_e + b2_e
  - multihead attention over the expert dim (E=8, H=8, HD=64) per token,
    additive float tril mask, output row `expert_id` only.

Strategy: data-parallel over the 4096 tokens across 8 cores (512 tokens/core),
weights replicated. Only attention-output row `expert_id` is needed, so only
q for that one expert is computed; each other expert's k/v are consumed
immediately after their projection (scores -> exp -> unnormalized ctx
accumulation on DVE, overlapped with the next expert's FFN on the PE).
Matmuls run in bf16 with fp32 PSUM accumulation.
"""

import sys

for _p in ("/opt/trn_rl_repo", "/root/.axon_site/_ro/trn_rl_repo"):
    if _p not in sys.path:
        sys.path.insert(0, _p)

import numpy as np
import ml_dtypes

import concourse.bass as bass
import concourse.mybir as mybir
import concourse.tile as tile
from concourse import bacc
from concourse.masks import make_identity
from concourse.bass_utils import run_bass_kernel_spmd

BF16 = mybir.dt.bfloat16
F32 = mybir.dt.float32
AF = mybir.ActivationFunctionType
ALU = mybir.AluOpType

# Problem shapes (hardcoded per contract).
E = 8
D = 512
H = 8
HD = 64
FF = 4 * D          # 2048
B, S = 4, 1024
N = B * S           # 4096 tokens
NCORES = 8
TOK = N // NCORES   # 512 tokens per core
P = 128
KC = D // P         # 4   (contraction chunks over D)
MC1 = FF // P       # 16  (output chunks of FFN1 / contraction chunks of FFN2)
NT = TOK // P       # 4   (token tiles per core)

_BUILD_CACHE: dict[int, "bass.Bass"] = {}


def _bcast_rows(ap: bass.AP, nparts: int) -> bass.AP:
    """[cols] DRAM vector -> [nparts, cols] partition-broadcast read AP."""
    return bass.AP(
        tensor=ap.tensor,
        offset=ap.offset,
        ap=[[0, nparts]] + [list(d) for d in ap.ap],
    )


def _bcast_free(ap: bass.AP, count: int) -> bass.AP:
    """Append a step-0 free dim of size `count` to an AP."""
    return bass.AP(
        tensor=ap.tensor,
        offset=ap.offset,
        ap=[list(d) for d in ap.ap] + [[0, count]],
    )


def build_kernel(
    expert_id: int, repeat: int | None = None, mode: str = "full",
    zero_bias: bool = False,
) -> bass.Bass:
    """Build the per-core NEFF. `repeat` wraps the whole body in a hardware
    loop; `mode` in {"full", "compute", "dma"} — bisection variants for
    timing (grading path uses repeat=None, mode="full")."""
    import contextlib

    nc = bacc.Bacc()

    # ---- DRAM I/O (host pre-arranged layouts; all DMAs read contiguous) ----
    xt = nc.dram_tensor("xt", [P, KC, TOK], BF16, kind="ExternalInput")
    w1 = nc.dram_tensor("w1", [E, P, KC, FF], BF16, kind="ExternalInput")
    w2 = nc.dram_tensor("w2", [E, P, MC1, D], BF16, kind="ExternalInput")
    wkT = nc.dram_tensor("wkT", [P, KC, D], BF16, kind="ExternalInput")
    wvT = nc.dram_tensor("wvT", [P, KC, D], BF16, kind="ExternalInput")
    wqT = nc.dram_tensor("wqT", [P, KC, D], BF16, kind="ExternalInput")  # pre-scaled 1/sqrt(HD)
    woT = nc.dram_tensor("woT", [P, KC, D], BF16, kind="ExternalInput")
    b1r = nc.dram_tensor("b1r", [E, P, MC1], F32, kind="ExternalInput")
    b2r = nc.dram_tensor("b2r", [E, P, KC], F32, kind="ExternalInput")
    bk = nc.dram_tensor("bk", [D], F32, kind="ExternalInput")
    bv = nc.dram_tensor("bv", [D], F32, kind="ExternalInput")
    bq = nc.dram_tensor("bq", [D], F32, kind="ExternalInput")  # pre-scaled 1/sqrt(HD)
    bo = nc.dram_tensor("bo", [D], F32, kind="ExternalInput")
    out = nc.dram_tensor("out", [NT, P, D], F32, kind="ExternalOutput")

    order = [expert_id] + [e for e in range(E) if e != expert_id]

    with tile.TileContext(nc) as tc:
        with (
            tc.tile_pool(name="const", bufs=1) as const_pool,
            tc.tile_pool(name="w1p", bufs=3) as w1_pool,
            tc.tile_pool(name="w2p", bufs=3) as w2_pool,
            tc.tile_pool(name="hid", bufs=MC1 + 2) as hid_pool,
            tc.tile_pool(name="aT", bufs=2 * KC) as aT_pool,
            tc.tile_pool(name="qkv", bufs=4) as qkv_pool,
            tc.tile_pool(name="attn", bufs=4) as attn_pool,
            tc.tile_pool(name="tmp", bufs=4) as tmp_pool,
            tc.tile_pool(name="outp", bufs=2) as out_pool,
            tc.tile_pool(name="psmm", bufs=6, space="PSUM") as psum_mm,
            tc.tile_pool(name="pstr", bufs=2, space="PSUM") as psum_tr,
        ):
            do_compute = mode != "dma"
            hoist = mode == "compute"  # loads outside the timing loop

            fixed = {}
            if hoist:
                fixed["w1"] = w1_pool.tile([P, KC, FF], BF16, tag="w1", name="w1fix")
                nc.sync.dma_start(out=fixed["w1"], in_=w1[0])
                fixed["w2"] = w2_pool.tile([P, MC1, D], BF16, tag="w2", name="w2fix")
                nc.sync.dma_start(out=fixed["w2"], in_=w2[0])
                fixed["b1"] = w1_pool.tile([P, MC1], F32, tag="b1", name="b1fix")
                nc.sync.dma_start(out=fixed["b1"], in_=b1r[0])
                fixed["b2"] = w2_pool.tile([P, KC], F32, tag="b2", name="b2fix")
                nc.sync.dma_start(out=fixed["b2"], in_=b2r[0])

            loop_ctx = (
                tc.For_i(0, repeat, 1, hint_engines=(
                    mybir.EngineType.PE, mybir.EngineType.Activation,
                    mybir.EngineType.DVE, mybir.EngineType.SP,
                ))
                if repeat else contextlib.nullcontext()
            )
            with loop_ctx:
                self_body(
                    nc, tc, expert_id, mode, fixed, zero_bias,
                    xt, w1, w2, wkT, wvT, wqT, woT, b1r, b2r, bk, bv, bq, bo, out,
                    const_pool, w1_pool, w2_pool, hid_pool, aT_pool, qkv_pool,
                    attn_pool, tmp_pool, out_pool, psum_mm, psum_tr,
                )

    nc.finalize()
    return nc


def self_body(
    nc, tc, expert_id, mode, fixed, zero_bias,
    xt, w1, w2, wkT, wvT, wqT, woT, b1r, b2r, bk, bv, bq, bo, out,
    const_pool, w1_pool, w2_pool, hid_pool, aT_pool, qkv_pool,
    attn_pool, tmp_pool, out_pool, psum_mm, psum_tr,
):
    do_compute = mode != "dma"
    order = [expert_id] + [e for e in range(E) if e != expert_id]

    if mode == "dma":
        # DMA-only bisection variant: stream all expert weights, minimal
        # consumer, write dummy output.
        last = None
        for e in range(E):
            w1_t = w1_pool.tile([P, KC, FF], BF16, tag="w1", name=f"w1d{e}")
            nc.sync.dma_start(out=w1_t, in_=w1[e])
            w2_t = w2_pool.tile([P, MC1, D], BF16, tag="w2", name=f"w2d{e}")
            nc.sync.dma_start(out=w2_t, in_=w2[e])
            last = (w1_t, w2_t)
        for t in range(NT):
            o_sb = out_pool.tile([P, D], F32, tag="o", name=f"od{t}")
            nc.scalar.copy(o_sb, last[t % 2][:, 0, 0:D])
            nc.sync.dma_start(out=out[t], in_=o_sb)
        return

    if True:
        if True:
            # ---- constants / global loads (xt first: FFN1 needs it) ----
            xt_sb = const_pool.tile([P, KC, TOK], BF16, tag="xt")
            nc.sync.dma_start(out=xt_sb[:, 0:2, :], in_=xt[:, 0:2, :])
            nc.sync.dma_start(out=xt_sb[:, 2:4, :], in_=xt[:, 2:4, :])

            def load_w1(w1_t, e):
                for kc in range(KC):
                    nc.sync.dma_start(out=w1_t[:, kc, :], in_=w1[e, :, kc, :])

            def load_w2(w2_t, e):
                for g in range(4):
                    nc.sync.dma_start(
                        out=w2_t[:, 4 * g:4 * g + 4, :], in_=w2[e, :, 4 * g:4 * g + 4, :]
                    )

            first_w = {}
            if not fixed:
                ei0 = order[0]
                first_w["w1"] = w1_pool.tile([P, KC, FF], BF16, tag="w1", name="w1_0")
                load_w1(first_w["w1"], ei0)
                first_w["w2"] = w2_pool.tile([P, MC1, D], BF16, tag="w2", name="w2_0")
                load_w2(first_w["w2"], ei0)
                if not zero_bias:
                    first_w["b1"] = w1_pool.tile([P, MC1], F32, tag="b1", name="b1_0")
                    nc.sync.dma_start(out=first_w["b1"], in_=b1r[ei0])
                    first_w["b2"] = w2_pool.tile([P, KC], F32, tag="b2", name="b2_0")
                    nc.sync.dma_start(out=first_w["b2"], in_=b2r[ei0])
                else:
                    first_w["b1"] = first_w["b2"] = None

            wk_sb = const_pool.tile([P, KC, D], BF16, tag="wk")
            wv_sb = const_pool.tile([P, KC, D], BF16, tag="wv")
            wq_sb = const_pool.tile([P, KC, D], BF16, tag="wq")
            wo_sb = const_pool.tile([P, KC, D], BF16, tag="wo")
            nc.sync.dma_start(out=wk_sb, in_=wkT[:, :, :])
            nc.sync.dma_start(out=wv_sb, in_=wvT[:, :, :])
            nc.sync.dma_start(out=wq_sb, in_=wqT[:, :, :])
            nc.sync.dma_start(out=wo_sb, in_=woT[:, :, :])

            if not zero_bias:
                bk_rep = const_pool.tile([P, D], F32, tag="bkr")
                bv_rep = const_pool.tile([P, D], F32, tag="bvr")
                bq_rep = const_pool.tile([P, D], F32, tag="bqr")
                bo_rep = const_pool.tile([P, D], F32, tag="bor")
                nc.sync.dma_start(out=bk_rep, in_=_bcast_rows(bk[:], P))
                nc.sync.dma_start(out=bv_rep, in_=_bcast_rows(bv[:], P))
                nc.sync.dma_start(out=bq_rep, in_=_bcast_rows(bq[:], P))
                nc.sync.dma_start(out=bo_rep, in_=_bcast_rows(bo[:], P))

            ident = const_pool.tile([P, P], BF16, tag="ident")
            make_identity(nc, ident)

            # persistent attention state, one per token tile
            exps = [const_pool.tile([P, H, E], F32, tag=f"exps{t}", name=f"exps{t}") for t in range(NT)]
            ctx = [const_pool.tile([P, H, HD], F32, tag=f"ctx{t}", name=f"ctx{t}") for t in range(NT)]
            q_sb = [const_pool.tile([P, D], BF16, tag=f"q{t}", name=f"q{t}") for t in range(NT)]

            # ---- expert loop (expert_id first: its projection produces q) ----
            for ei, e in enumerate(order):
                if fixed:
                    w1_t, w2_t = fixed["w1"], fixed["w2"]
                    b1_t, b2_t = fixed["b1"], fixed["b2"]
                elif ei == 0:
                    w1_t, w2_t = first_w["w1"], first_w["w2"]
                    b1_t, b2_t = first_w["b1"], first_w["b2"]
                else:
                    w1_t = w1_pool.tile([P, KC, FF], BF16, tag="w1", name=f"w1_{ei}")
                    load_w1(w1_t, e)
                    w2_t = w2_pool.tile([P, MC1, D], BF16, tag="w2", name=f"w2_{ei}")
                    load_w2(w2_t, e)
                    if not zero_bias:
                        b1_t = w1_pool.tile([P, MC1], F32, tag="b1", name=f"b1_{ei}")
                        nc.sync.dma_start(out=b1_t, in_=b1r[e])
                        b2_t = w2_pool.tile([P, KC], F32, tag="b2", name=f"b2_{ei}")
                        nc.sync.dma_start(out=b2_t, in_=b2r[e])
                    else:
                        b1_t = b2_t = None

                # FFN1: hiddenT[m] = relu(W1_e[:, m].T-chunks @ xT + b1)
                hid = []
                for m in range(MC1):
                    ps = psum_mm.tile([P, TOK], F32, tag="ps", bufs=6, name=f"psA{m}")
                    for kc in range(KC):
                        nc.tensor.matmul(
                            ps,
                            w1_t[:, kc, m * P:(m + 1) * P],
                            xt_sb[:, kc, :],
                            start=(kc == 0),
                            stop=(kc == KC - 1),
                        )
                    h_t = hid_pool.tile([P, TOK], BF16, tag="hid")
                    if zero_bias:
                        nc.scalar.activation(h_t, ps, AF.Relu)
                    else:
                        nc.scalar.activation(h_t, ps, AF.Relu, bias=b1_t[:, m:m + 1])
                    hid.append(h_t)

                # FFN2: aT[mc] = W2_e-chunks @ hiddenT + b2
                aT = []
                for mc in range(KC):
                    ps = psum_mm.tile([P, TOK], F32, tag="ps", bufs=6)
                    for k in range(MC1):
                        nc.tensor.matmul(
                            ps,
                            w2_t[:, k, mc * P:(mc + 1) * P],
                            hid[k],
                            start=(k == 0),
                            stop=(k == MC1 - 1),
                        )
                    a_t = aT_pool.tile([P, TOK], BF16, tag="aT")
                    if zero_bias:
                        nc.scalar.copy(a_t, ps)
                    else:
                        nc.scalar.activation(a_t, ps, AF.Identity, bias=b2_t[:, mc:mc + 1])
                    aT.append(a_t)

                # attention projections + incremental score/ctx per token tile
                maskval = 1.0 if e <= expert_id else 0.0
                for t in range(NT):
                    tsl = slice(t * P, (t + 1) * P)

                    if e == expert_id:
                        ps_q = psum_mm.tile([P, D], F32, tag="ps", bufs=6)
                        for kc in range(KC):
                            nc.tensor.matmul(
                                ps_q, aT[kc][:, tsl], wq_sb[:, kc, :],
                                start=(kc == 0), stop=(kc == KC - 1),
                            )
                        if zero_bias:
                            nc.scalar.copy(q_sb[t], ps_q)
                        else:
                            nc.vector.tensor_add(q_sb[t], ps_q, bq_rep)

                    ps_k = psum_mm.tile([P, D], F32, tag="ps", bufs=6)
                    for kc in range(KC):
                        nc.tensor.matmul(
                            ps_k, aT[kc][:, tsl], wk_sb[:, kc, :],
                            start=(kc == 0), stop=(kc == KC - 1),
                        )
                    k_sb = qkv_pool.tile([P, D], BF16, tag="k")
                    if zero_bias:
                        # last expert's drains go to ACT: shortens the DVE tail
                        if ei == E - 1:
                            nc.scalar.copy(k_sb, ps_k)
                        else:
                            nc.vector.tensor_copy(k_sb, ps_k)
                    else:
                        nc.vector.tensor_add(k_sb, ps_k, bk_rep)

                    ps_v = psum_mm.tile([P, D], F32, tag="ps", bufs=6)
                    for kc in range(KC):
                        nc.tensor.matmul(
                            ps_v, aT[kc][:, tsl], wv_sb[:, kc, :],
                            start=(kc == 0), stop=(kc == KC - 1),
                        )
                    v_sb = qkv_pool.tile([P, D], BF16, tag="v")
                    if zero_bias:
                        if ei == E - 1:
                            nc.scalar.copy(v_sb, ps_v)
                        else:
                            nc.vector.tensor_copy(v_sb, ps_v)
                    else:
                        nc.vector.tensor_add(v_sb, ps_v, bv_rep)

                    # scores for this expert: s[p, h] = sum_d q*k  (q pre-scaled)
                    prod = tmp_pool.tile([P, H, HD], BF16, tag="prod")
                    nc.vector.tensor_mul(
                        prod,
                        q_sb[t].rearrange("p (h d) -> p h d", d=HD),
                        k_sb.rearrange("p (h d) -> p h d", d=HD),
                    )
                    s_t = attn_pool.tile([P, H], F32, tag="s")
                    nc.vector.tensor_reduce(
                        s_t, prod, axis=mybir.AxisListType.X, op=ALU.add
                    )
                    # exp(s + mask[expert_id, e]) written into column e
                    nc.scalar.activation(
                        exps[t][:, :, e], s_t, AF.Exp, bias=maskval
                    )

                    # unnormalized ctx += exp_e (bcast over HD) * v_e
                    e_b = _bcast_free(exps[t][:, :, e], HD)
                    v3 = v_sb.rearrange("p (h d) -> p h d", d=HD)
                    if ei == 0:
                        nc.vector.tensor_mul(ctx[t], e_b, v3)
                    else:
                        cmul = tmp_pool.tile([P, H, HD], F32, tag="cmul")
                        nc.vector.tensor_mul(cmul, e_b, v3)
                        nc.vector.tensor_add(ctx[t], ctx[t], cmul)

            # ---- tail: normalize, transpose ctx, output projection ----
            ctxT = [const_pool.tile([P, TOK], BF16, tag=f"ctxT{kc}", name=f"ctxT{kc}") for kc in range(KC)]
            for t in range(NT):
                den = attn_pool.tile([P, H], F32, tag="den")
                nc.vector.tensor_reduce(
                    den, exps[t], axis=mybir.AxisListType.X, op=ALU.add
                )
                nc.vector.reciprocal(den, den)
                ctxn = tmp_pool.tile([P, H, HD], BF16, tag="ctxn")
                nc.vector.tensor_mul(ctxn, ctx[t], _bcast_free(den[:, :], HD))
                ctxn2 = ctxn.rearrange("p h d -> p (h d)")
                for kc in range(KC):
                    ps_t = psum_tr.tile([P, P], BF16, tag="pstr")
                    nc.tensor.transpose(ps_t, ctxn2[:, kc * P:(kc + 1) * P], ident)
                    nc.scalar.copy(ctxT[kc][:, t * P:(t + 1) * P], ps_t)

            for t in range(NT):
                ps_o = psum_mm.tile([P, D], F32, tag="ps", bufs=6)
                for kc in range(KC):
                    nc.tensor.matmul(
                        ps_o, ctxT[kc][:, t * P:(t + 1) * P], wo_sb[:, kc, :],
                        start=(kc == 0), stop=(kc == KC - 1),
                    )
                o_sb = out_pool.tile([P, D], F32, tag="o")
                if zero_bias:
                    nc.scalar.copy(o_sb, ps_o)
                else:
                    nc.vector.tensor_add(o_sb, ps_o, bo_rep)
                nc.sync.dma_start(out=out[t], in_=o_sb)


def _prep_inputs(x, W1, b1, W2, b2, Wq, bq, Wk, bk, Wv, bv, Wo, bo):
    """Host-side repack into the DMA-friendly layouts (shared across cores)."""
    bf = ml_dtypes.bfloat16
    f32 = np.float32
    scale = 1.0 / np.sqrt(np.float32(HD))

    w1h = np.ascontiguousarray(
        np.asarray(W1, f32).reshape(E, KC, P, FF).transpose(0, 2, 1, 3)
    ).astype(bf)
    w2h = np.ascontiguousarray(
        np.asarray(W2, f32).reshape(E, MC1, P, D).transpose(0, 2, 1, 3)
    ).astype(bf)

    def packT(w, s=1.0):
        # torch Linear weight [dout, din] -> lhs-friendly [P, KC, dout] of w.T
        wT = (np.asarray(w, f32).T * s).reshape(KC, P, D).transpose(1, 0, 2)
        return np.ascontiguousarray(wT).astype(bf)

    common = {
        "w1": w1h,
        "w2": w2h,
        "wkT": packT(Wk),
        "wvT": packT(Wv),
        "wqT": packT(Wq, scale),
        "woT": packT(Wo),
        "b1r": np.ascontiguousarray(
            np.asarray(b1, f32).reshape(E, MC1, P).transpose(0, 2, 1)
        ),
        "b2r": np.ascontiguousarray(
            np.asarray(b2, f32).reshape(E, KC, P).transpose(0, 2, 1)
        ),
        "bk": np.asarray(bk, f32),
        "bv": np.asarray(bv, f32),
        "bq": np.asarray(bq, f32) * scale,
        "bo": np.asarray(bo, f32),
    }

    xf = np.asarray(x, f32).reshape(N, D)
    in_maps = []
    for c in range(NCORES):
        xs = xf[c * TOK:(c + 1) * TOK]                      # [TOK, D]
        xTc = xs.T.reshape(KC, P, TOK).transpose(1, 0, 2)   # [P, KC, TOK]
        m = dict(common)
        m["xt"] = np.ascontiguousarray(xTc).astype(bf)
        in_maps.append(m)
    return in_maps


def _input_names(nc):
    names = set()
    for alloc in nc.m.functions[0].allocations:
        if isinstance(alloc, mybir.MemoryLocationSet) and alloc.kind == "ExternalInput":
            names.add(alloc.memorylocations[0].name)
    return names


def kernel(**inputs) -> np.ndarray:
    expert_id = int(np.asarray(inputs["expert_id"]))
    zb = all(
        not np.any(np.asarray(inputs[k], np.float32))
        for k in ("b1", "b2", "bq", "bk", "bv", "bo")
    )
    in_maps = _prep_inputs(
        inputs["x"], inputs["W1"], inputs["b1"], inputs["W2"], inputs["b2"],
        inputs["Wq"], inputs["bq"], inputs["Wk"], inputs["bk"],
        inputs["Wv"], inputs["bv"], inputs["Wo"], inputs["bo"],
    )
    key = (expert_id, zb)
    if key not in _BUILD_CACHE:
        _BUILD_CACHE[key] = build_kernel(expert_id, zero_bias=zb)
    nc = _BUILD_CACHE[key]

    want = _input_names(nc)
    in_maps = [{k: v for k, v in m.items() if k in want} for m in in_maps]
    res = run_bass_kernel_spmd(nc, in_maps, core_ids=list(range(NCORES)))
    shards = [res.results[c]["out"].reshape(TOK, D) for c in range(NCORES)]
    return np.concatenate(shards, axis=0).reshape(B, S, D).astype(np.float32)


if __name__ == "__main__":
    rng = np.random.default_rng(0)
    fake = {
        "x": rng.standard_normal((B, S, D), np.float32),
        "W1": rng.standard_normal((E, D, FF), np.float32) * 0.02,
        "b1": np.zeros((E, FF), np.float32),
        "W2": rng.standard_normal((E, FF, D), np.float32) * 0.02,
        "b2": np.zeros((E, D), np.float32),
        "Wq": rng.standard_normal((D, D), np.float32) * 0.02,
        "bq": np.zeros((D,), np.float32),
        "Wk": rng.standard_normal((D, D), np.float32) * 0.02,
        "bk": np.zeros((D,), np.float32),
        "Wv": rng.standard_normal((D, D), np.float32) * 0.02,
        "bv": np.zeros((D,), np.float32),
        "Wo": rng.standard_normal((D, D), np.float32) * 0.02,
        "bo": np.zeros((D,), np.float32),
        "expert_id": 3,
    }
    out = kernel(**fake)
    print("kernel out", out.shape, out.dtype)
